# revision 54
# baseline (speedup 1.0000x reference)
"""Trainium2 Bass kernel for nn_MultiHeadAttention_36112085025201.

Multi-head attention, B=2, S=4096, D=512, H=8 heads, Dh=64.
Sharding: 8 cores = 2 (batch) x 4 (head-pairs). Each core computes its
batch's attention for 2 heads plus that head-slice's contribution to the
output projection; the host sums the 4 partial projections per batch and
adds the fused (bv@wo + bo) bias row.

Per-core algorithm:
  - projections in bf16 (fp32 PSUM accum); q/k evicted to fp8e4 with
    per-partition bias, then DMA-remapped into the DoubleRow layout
    [32h+f%32, slot=f//32, s] so each scores matmul contracts all 64
    features as 2 packed k-tiles of 32 partitions (fp8 DoubleRow: 0.5
    cycles/row, 2x the bf16 stream rate).
  - v evicted to fp8e4 in a key-chunk-pair layout [128, h, m, slot, 65]
    (trailing ones column -> softmax denominator rides the PV matmul).
  - streaming attention per (1024-query block):
      per key chunk t: scores = DR-matmul -> PSUM f32;
      exp is split across BOTH ScalarE (native Exp -> fp8) and VectorE
      (Schraudolph bit-trick: n = round(A*s + B) as int8, bitcast fp8e4;
      softmax's invariance to a uniform scale makes the two engines'
      outputs interchangeable);
      PV accumulates 256 keys per DR matmul (vext [128,2,65] x e
      [128,2,512]).
  - normalize: denominator rows (partition 64 of the PV accumulators)
    copied to partitions 0/1, one batched approx-reciprocal, GPSIMD
    partition broadcast, one [64,1024] multiply per head -> onormT bf16.
  - out projection per 128-row tile in bf16; eviction is a plain copy on
    ScalarE (bias is added on the host), DMA straight out.
"""

import numpy as np
from contextlib import ExitStack

import ml_dtypes
import concourse.tile as tile
from concourse import bacc, mybir
from concourse.bass_utils import run_bass_kernel_spmd

# Problem constants (hardcoded per harness contract).
B, S, D = 2, 4096, 512
H, Dh = 8, 64
SCALE = Dh ** -0.5
N_CORES = 8
HL = 2                 # heads per core
CW = HL * Dh           # 128 local head columns per core
NK = D // 128          # 4 contraction chunks for projections
NSQ = S // 512         # 8 query blocks
NST = S // 128         # 32 key chunks (also 128-row output tiles)
NSP = NST // 2         # 16 key-chunk pairs (DoubleRow PV)
VW = Dh + 1            # v width incl. ones column

BF16 = mybir.dt.bfloat16
F32 = mybir.dt.float32
FP8 = mybir.dt.float8e4
I8 = mybir.dt.int8
I16 = mybir.dt.int16
EXP = mybir.ActivationFunctionType.Exp
MULT = mybir.AluOpType.mult
ADD = mybir.AluOpType.add
DR = mybir.MatmulPerfMode.DoubleRow

# Schraudolph exp constants (DVE bit-trick): value(n) = bitcast_fp8e4(n),
# n = round(A*s_raw + B) with s_raw the unscaled q.k dot product.
# fp8e4: n = 8*(log2(v)+7) => A = 8*log2(e)*SCALE, B = 56 + C,
# C = -0.458 centers the log-linear mantissa approximation.
LOG2E = 1.4426950408889634
SCH_C = -0.458
E_FP8 = True          # e/v in fp8e4 + DoubleRow PV (False: bf16 e/v, plain PV)
SCORES_FP8 = False    # q/k in fp8e4 + DoubleRow scores (False: bf16, as baseline)


def _exp_on_act(jp, m, sl, h):
    """Static ACT/DVE split of the 256 exp tiles: h1 on ScalarE, h0 on
    VectorE. h1's scores matmul completes last, so it gets the faster
    engine; with the denominator copies + out-proj evictions on ScalarE
    and the normalize muls + reciprocal on VectorE, this balances both
    engines at ~43us per query block."""
    return h == 1


def _build_body(ctx: ExitStack, tc: "tile.TileContext", io: dict, dbg: dict | None = None):
    nc = tc.nc
    xT, wq, wk, wv, wo = io["xT"], io["wq"], io["wk"], io["wv"], io["wo"]
    bq, bk, out = io["bq"], io["bk"], io["out"]

    const = ctx.enter_context(tc.tile_pool(name="const", bufs=1))
    persist = ctx.enter_context(tc.tile_pool(name="persist", bufs=1))

    QK8 = FP8 if SCORES_FP8 else BF16
    EV8 = FP8 if E_FP8 else BF16

    # Persistent SBUF arrays.
    xT_sb = [persist.tile([128, S], BF16, tag=f"xT{k}", name=f"xT{k}") for k in range(NK)]
    if SCORES_FP8:
        # DoubleRow layout: head h on partitions 32h..32h+31, feature f at
        # (partition 32h + f%32, slot f//32).
        qdr = persist.tile([64, 2, S], FP8, tag="qdr")
        kdr = persist.tile([64, 2, S], FP8, tag="kdr")
    else:
        qdr = persist.tile([128, S], BF16, tag="qdr")
        kdr = persist.tile([128, S], BF16, tag="kdr")
    # v in key-chunk-pair layout: [128 keys, h, pair m, slot, cols].
    # DoubleRow forbids column tiling (stationary must span 128 columns),
    # so in fp8 mode each head's stationary is [v(64) | ones(1) | zeros(63)]:
    # the PV output rows 0..63 are the head's attention output, row 64 the
    # softmax denominator, rows 65..127 zero.
    VWX = 128 if E_FP8 else VW
    vext = persist.tile([128, HL, NSP, 2, VWX], EV8, tag="vext")
    onormT = persist.tile([128, S], BF16, tag="onormT")

    wq_sb = [const.tile([128, CW], BF16, tag=f"wq{k}", name=f"wq{k}") for k in range(NK)]
    wk_sb = [const.tile([128, CW], BF16, tag=f"wk{k}", name=f"wk{k}") for k in range(NK)]
    wv_sb = [const.tile([128, CW], BF16, tag=f"wv{k}", name=f"wv{k}") for k in range(NK)]
    wo_sb = const.tile([128, D], BF16, tag="wo")
    bq_sb = const.tile([CW, 1], F32, tag="bq")
    bk_sb = const.tile([CW, 1], F32, tag="bk")

    # Input DMAs: only what the k-projection needs first (wk, bk, then xT
    # in consumption order); the remaining weights ride behind the first
    # xT blocks so the PE starts ~4us earlier.
    for k in range(NK):
        nc.sync.dma_start(wk_sb[k][:], wk[128 * k:128 * (k + 1), :])
    nc.sync.dma_start(bk_sb[:], bk[:, :])
    for jp in range(NSQ // 2):
        for k in range(NK):
            nc.sync.dma_start(xT_sb[k][:, 1024 * jp:1024 * (jp + 1)],
                              xT[128 * k:128 * (k + 1), 1024 * jp:1024 * (jp + 1)])
        if jp == 0:
            for k in range(NK):
                nc.sync.dma_start(wq_sb[k][:], wq[128 * k:128 * (k + 1), :])
                nc.sync.dma_start(wv_sb[k][:], wv[128 * k:128 * (k + 1), :])
            nc.sync.dma_start(wo_sb[:], wo[:, :])
            nc.sync.dma_start(bq_sb[:], bq[:, :])

    # PSUM pools (8 banks): pmm 2x[128,1024] = 4 banks, pacc 2x[128,1024] = 4.
    pmm = ctx.enter_context(tc.tile_pool(name="pmm", bufs=2, space="PSUM"))
    pacc = ctx.enter_context(tc.tile_pool(name="pacc", bufs=1, space="PSUM"))

    stagep = ctx.enter_context(tc.tile_pool(name="stagep", bufs=2))
    expp = ctx.enter_context(tc.tile_pool(name="expp", bufs=3))
    rp = ctx.enter_context(tc.tile_pool(name="rp", bufs=4))
    outp = ctx.enter_context(tc.tile_pool(name="outp", bufs=3))

    # vext: zero the pad, set the ones column; v copies fill cols 0..63.
    if E_FP8:
        nc.vector.memset(vext[:], 0.0)
        nc.vector.memset(vext[:, :, :, :, Dh:Dh + 1], 1.0)
    else:
        nc.vector.memset(vext[:], 1.0)

    # Phase A/B: projections. k first (phase C needs all of kT), then q and
    # v interleaved: q uses the pmm PSUM pool, v the (otherwise idle) pacc
    # pool, so their matmuls fill each other's eviction gaps on the PE.
    def qk_proj_block(w_sb, b_sb, dst, jp):
        ps = pmm.tile([128, 1024], F32, tag="mm")
        for k in range(NK):
            for jj in range(2):
                nc.tensor.matmul(ps[:, 512 * jj:512 * (jj + 1)], w_sb[k][:],
                                 xT_sb[k][:, 1024 * jp + 512 * jj:1024 * jp + 512 * (jj + 1)],
                                 start=(k == 0), stop=(k == NK - 1))
        if SCORES_FP8:
            st = stagep.tile([128, 1024], FP8, tag="stage")
            nc.vector.tensor_scalar_add(st[:], ps[:], b_sb[:])
            for h in range(HL):
                for i in range(2):
                    nc.sync.dma_start(
                        dst[32 * h:32 * (h + 1), i, 1024 * jp:1024 * (jp + 1)],
                        st[64 * h + 32 * i:64 * h + 32 * (i + 1), :])
        else:
            nc.vector.tensor_scalar_add(dst[:, 1024 * jp:1024 * (jp + 1)], ps[:], b_sb[:])

    def v_chunk(tp):
        # v projection in normal orientation [s, c], split per (head,
        # chunk-pair, slot) into vext (ones column preset above).
        ps = pmm.tile([128, 1024], F32, tag="mm", name=f"vps{tp}")
        for tt in range(2):
            t = 2 * tp + tt
            for k in range(NK):
                nc.tensor.matmul(ps[:, 512 * tt:512 * tt + CW],
                                 xT_sb[k][:, 128 * t:128 * (t + 1)], wv_sb[k][:],
                                 start=(k == 0), stop=(k == NK - 1))
        for tt in range(2):
            nc.vector.tensor_copy(
                vext[:, :, tp, tt, 0:Dh],
                ps[:, 512 * tt:512 * tt + CW].rearrange("p (h c) -> p h c", h=HL))

    for jp in range(NSQ // 2):
        qk_proj_block(wk_sb, bk_sb, kdr, jp)
    for jp in range(NSQ // 2):
        qk_proj_block(wq_sb, bq_sb, qdr, jp)
        for tp in range(4 * jp, 4 * jp + 4):
            v_chunk(tp)

    # Phase C: streaming attention + interleaved output projection.
    def out_proj_tile(jp_, st_):
        sq0 = 1024 * jp_ + 128 * st_
        pf = pmm.tile([128, 1024], F32, tag="mm", name="pf")
        nc.tensor.matmul(pf[:, 0:512], onormT[:, sq0:sq0 + 128], wo_sb[:],
                         start=True, stop=True)
        ob = outp.tile([128, 512], BF16, tag="ob")
        # Alternate the PSUM eviction between ScalarE and VectorE so the
        # pf buffers recycle twice as fast through the boundary block.
        if st_ % 2 == 0:
            nc.scalar.copy(ob[:], pf[:, 0:512])
        else:
            nc.vector.tensor_copy(ob[:], pf[:, 0:512])
        nc.sync.dma_start(out[sq0:sq0 + 128, :], ob[:])

    def out_proj_prev(jp_):
        for st_ in range(8):
            out_proj_tile(jp_, st_)

    sch_a = float(SCALE * 8.0 * LOG2E)
    sch_b = float(56.0 + SCH_C)
    sch_a16 = float(SCALE * 128.0 * LOG2E)
    sch_b16 = float(16256.0 + 16.0 * SCH_C)

    for jp in range(NSQ // 2):
        j0 = 2 * jp
        po = {h: pacc.tile([VWX, 1024], F32, tag=f"acc{h}", name=f"po{h}")
              for h in range(HL)}

        # PV for pair m-1 emitted between the scores of pair m (h0 after
        # slot 0, h1 after slot 1), so the PE has independent work queued
        # while the exp evictions free the scores PSUM buffers.
        def emit_pv(e_prev, m_prev, heads=(0, 1)):
            for h in heads:
                for jj in range(2):
                    if E_FP8:
                        nc.tensor.matmul(po[h][:, 512 * jj:512 * (jj + 1)],
                                         vext[:, h, m_prev, :, :],
                                         e_prev[h][:, :, 512 * jj:512 * (jj + 1)],
                                         start=(m_prev == 0), stop=(m_prev == NSP - 1),
                                         perf_mode=DR)
                    else:
                        for sl in range(2):
                            nc.tensor.matmul(po[h][:, 512 * jj:512 * (jj + 1)],
                                             vext[:, h, m_prev, sl, :],
                                             e_prev[h][:, sl, 512 * jj:512 * (jj + 1)],
                                             start=(m_prev == 0 and sl == 0),
                                             stop=(m_prev == NSP - 1 and sl == 1))

        e_hist = []
        for m in range(NSP):
            e_cur = {h: expp.tile([128, 2, 1024], EV8, tag="e", bufs=6, name=f"e{h}")
                     for h in range(HL)}
            for sl in range(2):
                t = 2 * m + sl
                # Alternate heads between consecutive matmuls: h0/h1 sit on
                # disjoint PE row groups, so adjacent MMs co-execute.
                s = {h: pmm.tile([128, 1024], F32, tag="mm", name=f"s{h}")
                     for h in range(HL)}
                for jj in range(2):
                    for h in range(HL):
                        if SCORES_FP8:
                            nc.tensor.matmul(s[h][:, 512 * jj:512 * (jj + 1)],
                                             kdr[32 * h:32 * (h + 1), :, 128 * t:128 * (t + 1)],
                                             qdr[32 * h:32 * (h + 1), :,
                                                 512 * (j0 + jj):512 * (j0 + jj + 1)],
                                             start=True, stop=True, perf_mode=DR)
                        else:
                            nc.tensor.matmul(s[h][:, 512 * jj:512 * (jj + 1)],
                                             kdr[Dh * h:Dh * (h + 1), 128 * t:128 * (t + 1)],
                                             qdr[Dh * h:Dh * (h + 1),
                                                 512 * (j0 + jj):512 * (j0 + jj + 1)],
                                             start=True, stop=True)
                # PV deferred by TWO pairs (the jp-boundary normalize chain
                # gets ~2 extra score iterations of slack). high_priority
                # makes the PV pair outrank the next slot's scores in the
                # scheduler's ready-heap, so it lands in the PE window where
                # the scores still wait on the exp evictions.
                if m >= 2:
                    with tc.high_priority(offset=40):
                        emit_pv(e_hist[m - 2], m - 2, heads=(sl,))
                for h in range(HL):
                    dst = e_cur[h][:, sl, :]
                    if _exp_on_act(jp, m, sl, h):
                        nc.scalar.activation(dst, s[h][:], EXP, scale=float(SCALE))
                    elif E_FP8:
                        nc.vector.tensor_scalar(dst.bitcast(I8), s[h][:],
                                                sch_a, sch_b, op0=MULT, op1=ADD)
                    else:
                        nc.vector.tensor_scalar(dst.bitcast(I16), s[h][:],
                                                sch_a16, sch_b16, op0=MULT, op1=ADD)
            e_hist.append(e_cur)
        emit_pv(e_hist[NSP - 2], NSP - 2)
        emit_pv(e_hist[NSP - 1], NSP - 1)
        if jp > 0:
            out_proj_prev(jp - 1)

        # Normalize: denominator rows -> partition 0 side by side (GPSIMD
        # broadcast and the approx-reciprocal ucode require partition-0
        # bases), reciprocal, per-head broadcast + multiply. The two
        # copies run on different engines in parallel.
        dt_ = rp.tile([1, HL * 1024], F32, tag="den")
        nc.scalar.copy(dt_[0:1, 0:1024], po[0][Dh:VW, :])
        nc.vector.tensor_copy(dt_[0:1, 1024:2048], po[1][Dh:VW, :])
        r = rp.tile([1, HL * 1024], F32, tag="r")
        # Per-head reciprocal halves: h0's broadcast+multiply chain starts
        # ~1us earlier than a single fused [1,2048] reciprocal would allow.
        for h in range(HL):
            nc.vector.reciprocal_approx_fast(r[0:1, 1024 * h:1024 * (h + 1)],
                                             dt_[0:1, 1024 * h:1024 * (h + 1)])
            rb = rp.tile([Dh, 1024], F32, tag="rb")
            nc.gpsimd.partition_broadcast(rb[:], r[0:1, 1024 * h:1024 * (h + 1)])
            nc.vector.tensor_mul(onormT[Dh * h:Dh * (h + 1), 1024 * jp:1024 * (jp + 1)],
                                 po[h][0:Dh, :], rb[:])
        if jp == NSQ // 2 - 1:
            out_proj_prev(jp)

    if dbg:
        for name, sb in (("onormT", onormT),):
            if name in dbg:
                nc.sync.dma_start(dbg[name][:, :], sb[:])


def build_nc():
    nc = bacc.Bacc("TRN2", target_bir_lowering=False, debug=False,
                   enable_asserts=False, num_devices=N_CORES)
    io = {
        "xT": nc.dram_tensor("xT", [D, S], BF16, kind="ExternalInput").ap(),
        "wq": nc.dram_tensor("wq", [D, CW], BF16, kind="ExternalInput").ap(),
        "wk": nc.dram_tensor("wk", [D, CW], BF16, kind="ExternalInput").ap(),
        "wv": nc.dram_tensor("wv", [D, CW], BF16, kind="ExternalInput").ap(),
        "wo": nc.dram_tensor("wo", [CW, D], BF16, kind="ExternalInput").ap(),
        "bq": nc.dram_tensor("bq", [CW, 1], F32, kind="ExternalInput").ap(),
        "bk": nc.dram_tensor("bk", [CW, 1], F32, kind="ExternalInput").ap(),
        "out": nc.dram_tensor("out", [S, D], BF16, kind="ExternalOutput").ap(),
    }
    with tile.TileContext(nc) as tc, ExitStack() as ctx:
        _build_body(ctx, tc, io)
    nc.compile()
    return nc


def make_in_maps(x, wq, bq, wk, bk, wv, bv, wo, bo):
    """Shard the full inputs across the 8 cores (host-side marshalling)."""
    bf16 = ml_dtypes.bfloat16
    in_maps = []
    for c in range(N_CORES):
        b, hp = divmod(c, 4)
        cs = slice(CW * hp, CW * (hp + 1))
        xT = np.ascontiguousarray(x[b].T).astype(bf16)
        in_maps.append({
            "xT": xT,
            "wq": np.ascontiguousarray(wq[:, cs]).astype(bf16),
            "wk": np.ascontiguousarray(wk[:, cs]).astype(bf16),
            "wv": np.ascontiguousarray(wv[:, cs]).astype(bf16),
            "wo": np.ascontiguousarray(wo[cs, :]).astype(bf16),
            "bq": np.ascontiguousarray(bq[cs].reshape(CW, 1)).astype(np.float32),
            "bk": np.ascontiguousarray(bk[cs].reshape(CW, 1)).astype(np.float32),
        })
    return in_maps


_CACHE = {}


def _get_nc():
    if "nc" not in _CACHE:
        _CACHE["nc"] = build_nc()
    return _CACHE["nc"]


def run_sharded(nc, in_maps, **kwargs):
    return run_bass_kernel_spmd(nc, in_maps, core_ids=list(range(N_CORES)), **kwargs)


def gather(results, bias_row):
    out = np.zeros((B, S, D), np.float32)
    for c in range(N_CORES):
        out[c // 4] += results[c]["out"].astype(np.float32)
    out += bias_row[None, None, :]
    return out


def kernel(x, wq, bq, wk, bk, wv, bv, wo, bo):
    x, wq, bq, wk, bk, wv, bv, wo, bo = (
        np.asarray(a, np.float32) for a in (x, wq, bq, wk, bk, wv, bv, wo, bo))
    nc = _get_nc()
    in_maps = make_in_maps(x, wq, bq, wk, bk, wv, bv, wo, bo)
    res = run_sharded(nc, in_maps)
    bias_row = (bv.astype(np.float64) @ wo.astype(np.float64)
                + bo.astype(np.float64)).astype(np.float32)
    return gather(res.results, bias_row)
